# revision 1
# baseline (speedup 1.0000x reference)
"""GAT layer kernel for Trainium2, 8 NeuronCores, row-sharded.

Math (reference):
    H = x @ W + bias                      # [N, D]
    h1 = H @ phi[:D];  h2 = H @ phi[D:]   # [N, 1]
    S = leaky_relu(h1 + h2.T, 0.01)
    S = where((adj + I) == 0, -9e15, S)
    out = softmax(S, axis=1) @ H

Device strategy (per core, rows sharded 8 ways):
    - compute full V = x @ W + bias on the PE from host-transposed xT
    - per 128-row strip: c = adj*10200-200 (gpsimd), t = Lrelu(h2b + h1[p])
      (scalar engine, per-partition bias), u = min(t, c) (vector),
      P = exp(u) (scalar; masked cols underflow to exactly 0),
      PE-transpose P chunks, matmul PT.T @ [V | 1] accumulating
      [h_raw | rowsum] in PSUM.  No max-subtraction needed: |scores| <~ 20.
    - outputs are unnormalized h_raw + rowsum + V; the host adds the
      forced self-loop term e_i = exp(lrelu(h1_i+h2_i)) for rows with
      adj[i,i] == 0 and divides.  (Avoids core-dependent diagonal
      indexing on device; exact.)
"""
import sys

sys.path.insert(0, "/opt/trn_rl_repo")

from contextlib import ExitStack

import numpy as np

import concourse.bacc as bacc
import concourse.tile as tile
from concourse import mybir
from concourse.masks import make_identity
import concourse.bass as bass

FP32 = mybir.dt.float32
BF16 = mybir.dt.bfloat16
INT32 = mybir.dt.int32


def _install_ntff_hook_shim():
    """The trimmed antenv package lacks axon_hooks; provide it so
    run_bass_kernel_spmd(trace=True) can capture NTFF profiles."""
    import types

    try:
        from antenv.axon_hooks import get_axon_ntff_profile_hook  # noqa: F401

        return  # real module present
    except ImportError:
        pass
    try:
        import antenv
        from trn_agent_boot.trn_boot import _ntff_profile_via_ctypes

        mod = types.ModuleType("antenv.axon_hooks")
        mod._hook = _ntff_profile_via_ctypes("/opt/axon/libaxon_pjrt.so")
        mod.get_axon_ntff_profile_hook = lambda: mod._hook
        mod.set_axon_ntff_profile_hook = lambda h: setattr(mod, "_hook", h)
        sys.modules["antenv.axon_hooks"] = mod
        antenv.axon_hooks = mod
    except Exception:
        pass


_install_ntff_hook_shim()

N_TOTAL = 8192
N_CORES = 8
N_LOCAL = N_TOTAL // N_CORES
D = 128
K_IN = 256
CB = 2048  # column block for elementwise passes


def build_gat(
    n_local=N_LOCAL,
    n_total=N_TOTAL,
    d=D,
    k_in=K_IN,
    cb=CB,
    lrelu_mode="act",  # "act" (HW Lrelu) | "two_exp" (sim-checkable, exact)
    p_dtype=FP32,  # dtype of P/V fed to the output matmul
):
    assert n_local % 128 == 0 and n_total % cb == 0 and cb % 128 == 0
    n_strips = n_local // 128
    n_chunks = n_total // 128
    n_cb = n_total // cb
    group = min(512, cb)  # transpose/psum batch width
    gchunks = group // 128
    dc = d + 1  # V chunk width incl. ones column

    nc = bacc.Bacc()
    adj = nc.declare_dram_parameter("adj", [n_local, n_total], INT32, isOutput=False)
    xT = nc.declare_dram_parameter("xt", [k_in, n_total], FP32, isOutput=False)
    w = nc.declare_dram_parameter("w", [k_in, d], FP32, isOutput=False)
    biasd = nc.declare_dram_parameter("bias", [d], FP32, isOutput=False)
    h1d = nc.declare_dram_parameter("h1loc", [n_local], FP32, isOutput=False)
    h2d = nc.declare_dram_parameter("h2", [n_total], FP32, isOutput=False)
    hrawd = nc.declare_dram_parameter("h_raw", [n_local, d], FP32, isOutput=True)
    rsumd = nc.declare_dram_parameter("rsum", [n_local, 1], FP32, isOutput=True)
    vfulld = nc.declare_dram_parameter("vfull", [n_total, d], p_dtype, isOutput=True)

    def bcast(ap_1d, parts, n):
        return bass.AP(
            tensor=ap_1d.tensor, offset=ap_1d.offset, ap=[[0, parts], [1, n]]
        )

    with tile.TileContext(nc) as tc, ExitStack() as ctx:
        consts = ctx.enter_context(tc.tile_pool(name="consts", bufs=1))
        vpool = ctx.enter_context(tc.tile_pool(name="v", bufs=1))

        ident = consts.tile([128, 128], p_dtype)
        make_identity(nc, ident)

        wsb = consts.tile([128, 2, d], FP32)
        nc.sync.dma_start(out=wsb[:, 0, :], in_=w[0:128, :])
        nc.sync.dma_start(out=wsb[:, 1, :], in_=w[128:256, :])

        biasb = consts.tile([128, d], FP32)
        nc.sync.dma_start(out=biasb, in_=bcast(biasd[:], 128, d))

        h1sb = consts.tile([128, n_strips], FP32)
        h1ap = h1d[:]
        nc.sync.dma_start(
            out=h1sb,
            in_=bass.AP(
                tensor=h1ap.tensor, offset=h1ap.offset, ap=[[1, 128], [128, n_strips]]
            ),
        )
        h2b = consts.tile([128, n_total], FP32)
        nc.sync.dma_start(out=h2b, in_=bcast(h2d[:], 128, n_total))

        V = vpool.tile([128, n_chunks * dc], p_dtype)

        # ---------------- preamble: V = x @ W + bias ----------------
        with (
            tc.tile_pool(name="xt", bufs=1) as xpool,
            tc.tile_pool(name="hv", bufs=4, space="PSUM") as hvpool,
        ):
            xts = xpool.tile([128, 2, n_total], FP32)
            nc.sync.dma_start(out=xts[:, 0, :], in_=xT[0:128, :])
            nc.sync.dma_start(out=xts[:, 1, :], in_=xT[128:256, :])
            for r in range(n_chunks):
                hv = hvpool.tile([128, d], FP32)
                nc.tensor.matmul(
                    hv,
                    lhsT=xts[:, 0, r * 128 : (r + 1) * 128],
                    rhs=wsb[:, 0, :],
                    start=True,
                    stop=False,
                )
                nc.tensor.matmul(
                    hv,
                    lhsT=xts[:, 1, r * 128 : (r + 1) * 128],
                    rhs=wsb[:, 1, :],
                    start=False,
                    stop=True,
                )
                nc.vector.tensor_tensor(
                    out=V[:, r * dc : r * dc + d],
                    in0=hv,
                    in1=biasb,
                    op=mybir.AluOpType.add,
                )
                nc.sync.dma_start(
                    out=vfulld[r * 128 : (r + 1) * 128, :], in_=V[:, r * dc : r * dc + d]
                )
            for r in range(n_chunks):
                nc.vector.memset(V[:, r * dc + d : (r + 1) * dc], 1.0)

        # ---------------- main loop over strips ----------------
        adj_pool = ctx.enter_context(tc.tile_pool(name="adjp", bufs=3))
        c_pool = ctx.enter_context(tc.tile_pool(name="cp", bufs=2))
        t_pool = ctx.enter_context(tc.tile_pool(name="tp", bufs=2))
        p_pool = ctx.enter_context(tc.tile_pool(name="pp", bufs=2))
        pt_pool = ctx.enter_context(tc.tile_pool(name="ptp", bufs=3))
        out_pool = ctx.enter_context(tc.tile_pool(name="outp", bufs=2))
        hps_pool = ctx.enter_context(tc.tile_pool(name="hps", bufs=2, space="PSUM"))
        tps_pool = ctx.enter_context(tc.tile_pool(name="tps", bufs=3, space="PSUM"))

        for s in range(n_strips):
            hps = hps_pool.tile([128, dc], FP32)
            for icb in range(n_cb):
                cbs = icb * cb
                adjt = adj_pool.tile([128, cb], INT32)
                nc.sync.dma_start(
                    out=adjt, in_=adj[s * 128 : (s + 1) * 128, cbs : cbs + cb]
                )
                ct = c_pool.tile([128, cb], FP32)
                if lrelu_mode == "act":
                    # c = adj*10200 - 200 : +10000 unmasked / -200 masked
                    nc.gpsimd.tensor_scalar(
                        out=ct,
                        in0=adjt,
                        scalar1=10200.0,
                        scalar2=-200.0,
                        op0=mybir.AluOpType.mult,
                        op1=mybir.AluOpType.add,
                    )
                    tt = t_pool.tile([128, cb], FP32)
                    nc.scalar.activation(
                        tt,
                        h2b[:, cbs : cbs + cb],
                        mybir.ActivationFunctionType.Lrelu,
                        bias=h1sb[:, s : s + 1],
                        scale=1.0,
                        alpha=0.01,
                    )
                    nc.vector.tensor_tensor(
                        out=tt, in0=tt, in1=ct, op=mybir.AluOpType.min
                    )
                    pt = p_pool.tile([128, cb], p_dtype)
                    nc.scalar.activation(pt, tt, mybir.ActivationFunctionType.Exp)
                else:
                    # exact lrelu-free variant: u = min(s, adj*20200-20000)
                    # (unmasked: +200 >= s, masked: -20000), then
                    # P = max(exp(u), exp(0.01u)) == exp(lrelu(u)); masked
                    # cols give exp(-20000)=0 and exp(-200)=0.
                    nc.gpsimd.tensor_scalar(
                        out=ct,
                        in0=adjt,
                        scalar1=20200.0,
                        scalar2=-20000.0,
                        op0=mybir.AluOpType.mult,
                        op1=mybir.AluOpType.add,
                    )
                    tt = t_pool.tile([128, cb], FP32)
                    nc.scalar.activation(
                        tt,
                        h2b[:, cbs : cbs + cb],
                        mybir.ActivationFunctionType.Identity,
                        bias=h1sb[:, s : s + 1],
                        scale=1.0,
                    )
                    nc.vector.tensor_tensor(
                        out=tt, in0=tt, in1=ct, op=mybir.AluOpType.min
                    )
                    e1 = p_pool.tile([128, cb], FP32)
                    nc.scalar.activation(e1, tt, mybir.ActivationFunctionType.Exp)
                    e2 = t_pool.tile([128, cb], FP32)
                    nc.scalar.activation(
                        e2, tt, mybir.ActivationFunctionType.Exp, scale=0.01
                    )
                    pt = p_pool.tile([128, cb], p_dtype)
                    nc.vector.tensor_tensor(
                        out=pt, in0=e1, in1=e2, op=mybir.AluOpType.max
                    )

                for g in range(cb // group):
                    tps = tps_pool.tile([128, group], p_dtype)
                    for c4 in range(gchunks):
                        nc.tensor.transpose(
                            tps[:, c4 * 128 : (c4 + 1) * 128],
                            pt[:, g * group + c4 * 128 : g * group + (c4 + 1) * 128],
                            ident,
                        )
                    ptsb = pt_pool.tile([128, group], p_dtype)
                    if g % 2 == 0:
                        nc.scalar.copy(out=ptsb, in_=tps)
                    else:
                        nc.vector.tensor_copy(ptsb, tps)
                    for c4 in range(gchunks):
                        ch = cbs // 128 + g * gchunks + c4
                        nc.tensor.matmul(
                            hps,
                            lhsT=ptsb[:, c4 * 128 : (c4 + 1) * 128],
                            rhs=V[:, ch * dc : (ch + 1) * dc],
                            start=(ch == 0),
                            stop=(ch == n_chunks - 1),
                        )
            hsb = out_pool.tile([128, dc], FP32)
            nc.vector.tensor_copy(hsb, hps)
            nc.sync.dma_start(
                out=hrawd[s * 128 : (s + 1) * 128, :], in_=hsb[:, 0:d]
            )
            nc.sync.dma_start(
                out=rsumd[s * 128 : (s + 1) * 128, :], in_=hsb[:, d : d + 1]
            )

    nc.finalize()
    return nc


_NC_CACHE = {}


def _get_nc(key):
    if key not in _NC_CACHE:
        _NC_CACHE[key] = build_gat(
            n_local=key[0], n_total=key[1], cb=key[2], lrelu_mode=key[3],
            p_dtype=FP32 if key[4] == "fp32" else BF16,
        )
    return _NC_CACHE[key]


def _host_prep(adj, x, weight, bias, phi):
    d = weight.shape[1]
    x = np.asarray(x, dtype=np.float32)
    weight = np.asarray(weight, dtype=np.float32)
    bias = np.asarray(bias, dtype=np.float32)
    phi = np.asarray(phi, dtype=np.float32)
    xT = np.ascontiguousarray(x.T)
    w1 = (weight @ phi[:d, 0]).astype(np.float32)
    w2 = (weight @ phi[d:, 0]).astype(np.float32)
    b1 = np.float32(bias @ phi[:d, 0])
    b2 = np.float32(bias @ phi[d:, 0])
    h1 = (x @ w1 + b1).astype(np.float32)
    h2 = (x @ w2 + b2).astype(np.float32)
    return xT, weight, bias, h1, h2


def _host_post(adj, h1, h2, h_raw, rsum, vfull):
    sdiag = h1 + h2
    lr = np.where(sdiag >= 0, sdiag, np.float32(0.01) * sdiag).astype(np.float32)
    e = np.where(np.ascontiguousarray(np.diagonal(adj)) == 0, np.exp(lr), 0.0).astype(
        np.float32
    )
    h = (h_raw + e[:, None] * vfull.astype(np.float32)) / (rsum + e)[:, None]
    return h.astype(np.float32)


def run_gat(adj, x, weight, bias, phi, trace=False, trace_kwargs=None):
    """Returns (h, BassKernelResults)."""
    import os

    n, k_in = x.shape
    adj = np.asarray(adj)
    xT, weight, bias, h1, h2 = _host_prep(adj, x, weight, bias, phi)
    n_local = n // N_CORES
    pdt = os.environ.get("GAT_PDT", "fp32")
    cb = int(os.environ.get("GAT_CB", str(CB)))
    nc = _get_nc((n_local, n, cb, "act", pdt))

    from concourse.bass_utils import run_bass_kernel_spmd

    in_maps = []
    for c in range(N_CORES):
        sl = slice(c * n_local, (c + 1) * n_local)
        in_maps.append(
            {
                "adj": adj[sl],
                "xt": xT,
                "w": weight,
                "bias": bias,
                "h1loc": np.ascontiguousarray(h1[sl]),
                "h2": h2,
            }
        )
    kw = dict(trace_kwargs or {})
    res = run_bass_kernel_spmd(nc, in_maps, list(range(N_CORES)), trace=trace, **kw)
    h_raw = np.concatenate([res.results[c]["h_raw"] for c in range(N_CORES)], axis=0)
    rsum = np.concatenate(
        [res.results[c]["rsum"][:, 0] for c in range(N_CORES)], axis=0
    )
    vfull = res.results[0]["vfull"]
    return _host_post(adj, h1, h2, h_raw, rsum, vfull), res


def kernel(adj, x, weight, bias, phi):
    h, _ = run_gat(adj, x, weight, bias, phi)
    return h

